# revision 4
# baseline (speedup 1.0000x reference)
"""Trainium2 Bass kernel for nn_AttentionBlock (GroupNorm + spatial self-attention + residual).

Full inputs in, full outputs out. Internally: data-parallel over the batch dim
(B=8) across 8 NeuronCores; each core runs an identical Bass/Tile program on
one [C=256, N=4096] image.

Per-core layout choices:
  - q,k stored [c, n] (c on partitions, 2 chunks of 128)
  - v stored transposed [n, c] (computed directly by swapping matmul operand
    roles, no on-device transpose pass)
  - attention scores computed transposed: S'[j,i] = (K^T Q)[j,i], j on
    partitions, so the AV contraction over j runs as PSUM-accumulated matmuls
  - softmax denominator: DVE accumulation of exp tiles over j-tiles, then a
    ones-vector matmul to reduce the 128 partitions; normalization applied to
    A via a broadcast tile (ones outer-product matmul)
  - all big matmuls in bf16 (1 cycle/row on PE vs 4 for fp32); accumulation is
    always fp32 in PSUM. exp(s/16) runs on ACT straight out of PSUM with the
    1/sqrt(C) folded into the activation scale; no max-subtraction (scores are
    in [-8, 7] for this distribution, exp is safe in fp32).
"""

import sys

try:
    import concourse  # noqa: F401
except ImportError:
    sys.path.insert(0, "/opt/trn_rl_repo")

import numpy as np
import ml_dtypes

import concourse.bacc as bacc
import concourse.tile as tile
from concourse import mybir
from concourse.bass_utils import run_bass_kernel_spmd

F32 = mybir.dt.float32
BF16 = mybir.dt.bfloat16
AF = mybir.ActivationFunctionType
ALU = mybir.AluOpType
AX = mybir.AxisListType

C = 256          # channels
N = 4096         # spatial positions (64*64)
GROUPS = 32      # groupnorm groups -> 8 channels per group
EPS = 1e-5
SCALE = C ** -0.5
NSTRIPE = 8      # stripes over the spatial dim
SW = N // NSTRIPE  # 512
NJT = N // 128   # 32 j-tiles
GSIZE = (C // GROUPS) * N  # elements per group = 32768


def _emit(nc, tc, d):
    """Emit the per-core program. d: dict of DRAM tensor handles."""
    const = tc.alloc_tile_pool(name="const", bufs=1)

    # --- weights / params -> SBUF ---
    wq = []
    for t in range(2):
        w = const.tile([128, 768], BF16, tag=f"wq{t}", name=f"wq{t}")
        nc.sync.dma_start(w[:], d["wqkvT"][t * 128:(t + 1) * 128, :])
        wq.append(w)
    wp = []
    for t in range(2):
        w = const.tile([128, 256], BF16, tag=f"wp{t}", name=f"wp{t}")
        nc.sync.dma_start(w[:], d["wprojT"][t * 128:(t + 1) * 128, :])
        wp.append(w)
    vbb = const.tile([128, 256], BF16, tag="vbb")
    nc.sync.dma_start(vbb[:], d["vbias"][:])
    qkvb = const.tile([128, 4], F32, tag="qkvb")
    nc.sync.dma_start(qkvb[:], d["qkvb"][:])
    projb = const.tile([128, 2], F32, tag="projb")
    nc.sync.dma_start(projb[:], d["projb"][:])
    nw = const.tile([128, 2], F32, tag="nw")
    nc.sync.dma_start(nw[:], d["normw"][:])
    nb = const.tile([128, 2], F32, tag="nb")
    nc.sync.dma_start(nb[:], d["normb"][:])
    onesc = const.tile([128, 1], F32, tag="onesc")
    nc.sync.dma_start(onesc[:], d["onescol"][:])
    onesr = const.tile([1, 128], F32, tag="onesr")
    nc.sync.dma_start(onesr[:], d["onesrow"][:])
    gm = const.tile([128, 128], F32, tag="gm")
    nc.sync.dma_start(gm[:], d["gm"][:])

    # --- x -> SBUF (resident, also used for the residual at the end) ---
    xt = []
    for t in range(2):
        x_ = const.tile([128, N], F32, tag=f"x{t}", name=f"x{t}")
        nc.sync.dma_start(x_[:], d["x"][t * 128:(t + 1) * 128, :])
        xt.append(x_)

    # --- phase A: groupnorm stats -> per-channel scale/bias ---
    # stats cols: [sum(chunk0), sumsq(chunk0), sum(chunk1), sumsq(chunk1)]
    stats = const.tile([128, 4], F32, tag="stats")
    scl = const.tile([128, 2], F32, tag="scl")
    bia = const.tile([128, 2], F32, tag="bia")
    with tc.tile_pool(name="scratch", bufs=2) as scr, \
         tc.tile_pool(name="pa_ps", bufs=1, space="PSUM") as pa_ps:
        for t in range(2):
            nc.vector.reduce_sum(stats[:, 2 * t:2 * t + 1], xt[t][:], axis=AX.X)
            sq = scr.tile([128, N], F32, tag="sq")
            nc.scalar.activation(sq[:], xt[t][:], AF.Square,
                                 accum_out=stats[:, 2 * t + 1:2 * t + 2])
        gstats = pa_ps.tile([128, 4], F32, tag="gstats")
        nc.tensor.matmul(gstats[:], gm[:], stats[:], start=True, stop=True)

        mean = const.tile([128, 2], F32, tag="mean")
        ex2 = const.tile([128, 2], F32, tag="ex2")
        for t in range(2):
            nc.vector.tensor_scalar_mul(mean[:, t:t + 1], gstats[:, 2 * t:2 * t + 1], 1.0 / GSIZE)
            nc.vector.tensor_scalar_mul(ex2[:, t:t + 1], gstats[:, 2 * t + 1:2 * t + 2], 1.0 / GSIZE)
        m2 = const.tile([128, 2], F32, tag="m2")
        var = const.tile([128, 2], F32, tag="var")
        std = const.tile([128, 2], F32, tag="std")
        rstd = const.tile([128, 2], F32, tag="rstd")
        nc.vector.tensor_mul(m2[:], mean[:], mean[:])
        nc.vector.tensor_sub(var[:], ex2[:], m2[:])
        nc.vector.tensor_scalar_add(var[:], var[:], EPS)
        nc.scalar.activation(std[:], var[:], AF.Sqrt)
        nc.vector.reciprocal(rstd[:], std[:])
        nc.vector.tensor_mul(scl[:], nw[:], rstd[:])
        mscl = const.tile([128, 2], F32, tag="mscl")
        nc.vector.tensor_mul(mscl[:], mean[:], scl[:])
        nc.vector.tensor_sub(bia[:], nb[:], mscl[:])

    # --- phase B: h = x*scl+bia (bf16), q,k ([c,n]) and vT ([n,c]) ---
    qk = []
    for i in range(4):  # q0,q1,k0,k1
        t_ = const.tile([128, N], BF16, tag=f"qk{i}", name=f"qk{i}")
        qk.append(t_)
    vt = []
    for j in range(NJT):
        t_ = const.tile([128, 256], BF16, tag=f"vt{j}", name=f"vt{j}")
        vt.append(t_)

    with tc.tile_pool(name="hpool", bufs=3) as hp, \
         tc.tile_pool(name="pb_ps", bufs=3, space="PSUM") as pbp, \
         tc.tile_pool(name="pv_ps", bufs=3, space="PSUM") as pvp:
        for s in range(NSTRIPE):
            sl = slice(s * SW, (s + 1) * SW)
            hts = []
            for t in range(2):
                ht = hp.tile([128, SW], BF16, tag=f"h{t}", name=f"h{t}")
                nc.vector.tensor_scalar(ht[:], xt[t][:, sl], scl[:, t:t + 1],
                                        bia[:, t:t + 1], op0=ALU.mult, op1=ALU.add)
                hts.append(ht)
            for dt in range(4):
                ps = pbp.tile([128, SW], F32, tag="qkps", name="qkps")
                nc.tensor.matmul(ps[:], wq[0][:, dt * 128:(dt + 1) * 128], hts[0][:],
                                 start=True, stop=False)
                nc.tensor.matmul(ps[:], wq[1][:, dt * 128:(dt + 1) * 128], hts[1][:],
                                 start=False, stop=True)
                nc.vector.tensor_scalar_add(qk[dt][:, sl], ps[:], qkvb[:, dt:dt + 1])
            for n4 in range(4):
                jt = s * 4 + n4
                psv = pvp.tile([128, 256], F32, tag="vtps", name="vtps")
                nc.tensor.matmul(psv[:], hts[0][:, n4 * 128:(n4 + 1) * 128],
                                 wq[0][:, 512:768], start=True, stop=False)
                nc.tensor.matmul(psv[:], hts[1][:, n4 * 128:(n4 + 1) * 128],
                                 wq[1][:, 512:768], start=False, stop=True)
                nc.vector.tensor_add(vt[jt][:], psv[:], vbb[:])

    # --- phase C: attention + proj + residual, per i-stripe ---
    with tc.tile_pool(name="wpool", bufs=6) as wpo, \
         tc.tile_pool(name="raccp", bufs=2) as rp, \
         tc.tile_pool(name="misc", bufs=2) as mp, \
         tc.tile_pool(name="s_ps", bufs=2, space="PSUM") as sp, \
         tc.tile_pool(name="a_ps", bufs=2, space="PSUM") as apo, \
         tc.tile_pool(name="r_ps", bufs=1, space="PSUM") as rsp, \
         tc.tile_pool(name="b_ps", bufs=1, space="PSUM") as bpo, \
         tc.tile_pool(name="o_ps", bufs=2, space="PSUM") as opo:
        for ist in range(NSTRIPE):
            sl = slice(ist * SW, (ist + 1) * SW)
            racc = rp.tile([128, SW], F32, tag="racc")
            a_ps = [apo.tile([128, SW], F32, tag="aps", name="aps") for _ in range(2)]
            for jt in range(NJT):
                s_ps = sp.tile([128, SW], F32, tag="sps", name="sps")
                nc.tensor.matmul(s_ps[:], qk[2][:, jt * 128:(jt + 1) * 128], qk[0][:, sl],
                                 start=True, stop=False)
                nc.tensor.matmul(s_ps[:], qk[3][:, jt * 128:(jt + 1) * 128], qk[1][:, sl],
                                 start=False, stop=True)
                w_sb = wpo.tile([128, SW], BF16, tag="wsb", name="wsb")
                nc.scalar.activation(w_sb[:], s_ps[:], AF.Exp, scale=SCALE)
                if jt == 0:
                    nc.vector.tensor_copy(racc[:], w_sb[:])
                else:
                    nc.vector.tensor_add(racc[:], racc[:], w_sb[:])
                for ct in range(2):
                    nc.tensor.matmul(a_ps[ct][:], vt[jt][:, ct * 128:(ct + 1) * 128],
                                     w_sb[:], start=(jt == 0), stop=(jt == NJT - 1))
            rs_ps = rsp.tile([1, SW], F32, tag="rs")
            nc.tensor.matmul(rs_ps[:], onesc[:], racc[:], start=True, stop=True)
            rinv = mp.tile([1, SW], F32, tag="rinv")
            nc.vector.reciprocal(rinv[:], rs_ps[:])
            bc_ps = bpo.tile([128, SW], F32, tag="bc")
            nc.tensor.matmul(bc_ps[:], onesr[:], rinv[:], start=True, stop=True)
            bc_sb = mp.tile([128, SW], F32, tag="bcs")
            nc.vector.tensor_copy(bc_sb[:], bc_ps[:])
            a_sb = []
            for ct in range(2):
                t_ = mp.tile([128, SW], BF16, tag=f"asb{ct}", name=f"asb{ct}")
                nc.vector.tensor_mul(t_[:], a_ps[ct][:], bc_sb[:])
                a_sb.append(t_)
            for dt in range(2):
                o_ps = opo.tile([128, SW], F32, tag="ops", name="ops")
                nc.tensor.matmul(o_ps[:], wp[0][:, dt * 128:(dt + 1) * 128], a_sb[0][:],
                                 start=True, stop=False)
                nc.tensor.matmul(o_ps[:], wp[1][:, dt * 128:(dt + 1) * 128], a_sb[1][:],
                                 start=False, stop=True)
                o_sb = mp.tile([128, SW], F32, tag=f"osb{dt}", name=f"osb{dt}")
                nc.vector.scalar_tensor_tensor(o_sb[:], o_ps[:], projb[:, dt:dt + 1],
                                               xt[dt][:, sl], op0=ALU.add, op1=ALU.add)
                nc.sync.dma_start(d["out"][dt * 128:(dt + 1) * 128, sl], o_sb[:])

    const.release()


def build_program(repeat: int = 1):
    nc = bacc.Bacc("TRN2", target_bir_lowering=False, debug=False, num_devices=8)
    d = {
        "x": nc.declare_dram_parameter("x", [C, N], F32, isOutput=False),
        "wqkvT": nc.declare_dram_parameter("wqkvT", [C, 3 * C], BF16, isOutput=False),
        "wprojT": nc.declare_dram_parameter("wprojT", [C, C], BF16, isOutput=False),
        "qkvb": nc.declare_dram_parameter("qkvb", [128, 4], F32, isOutput=False),
        "vbias": nc.declare_dram_parameter("vbias", [128, 256], BF16, isOutput=False),
        "projb": nc.declare_dram_parameter("projb", [128, 2], F32, isOutput=False),
        "normw": nc.declare_dram_parameter("normw", [128, 2], F32, isOutput=False),
        "normb": nc.declare_dram_parameter("normb", [128, 2], F32, isOutput=False),
        "onescol": nc.declare_dram_parameter("onescol", [128, 1], F32, isOutput=False),
        "onesrow": nc.declare_dram_parameter("onesrow", [1, 128], F32, isOutput=False),
        "gm": nc.declare_dram_parameter("gm", [128, 128], F32, isOutput=False),
        "out": nc.declare_dram_parameter("out", [C, N], F32, isOutput=True),
    }
    with tile.TileContext(nc) as tc:
        if repeat == 1:
            _emit(nc, tc, d)
        else:
            def body(_i):
                _emit(nc, tc, d)
            tc.For_i_unrolled(0, repeat, 1, body, max_unroll=1)
    nc.compile()
    return nc


def make_in_maps(x, norm_w, norm_b, qkv_w, qkv_b, proj_w, proj_b):
    x = np.asarray(x, np.float32)
    B = x.shape[0]
    qkv_w = np.asarray(qkv_w, np.float32)
    qkv_b = np.asarray(qkv_b, np.float32)
    proj_w = np.asarray(proj_w, np.float32)
    proj_b = np.asarray(proj_b, np.float32)
    shared = {
        "wqkvT": np.ascontiguousarray(qkv_w.T).astype(ml_dtypes.bfloat16),
        "wprojT": np.ascontiguousarray(proj_w.T).astype(ml_dtypes.bfloat16),
        "qkvb": np.ascontiguousarray(qkv_b[:512].reshape(4, 128).T),
        "vbias": np.tile(qkv_b[512:].reshape(1, 256), (128, 1)).astype(ml_dtypes.bfloat16),
        "projb": np.ascontiguousarray(proj_b.reshape(2, 128).T),
        "normw": np.ascontiguousarray(np.asarray(norm_w, np.float32).reshape(2, 128).T),
        "normb": np.ascontiguousarray(np.asarray(norm_b, np.float32).reshape(2, 128).T),
        "onescol": np.ones((128, 1), np.float32),
        "onesrow": np.ones((1, 128), np.float32),
        "gm": (np.arange(128)[:, None] // 8 == np.arange(128)[None, :] // 8).astype(np.float32),
    }
    return [dict(shared, x=np.ascontiguousarray(x[b].reshape(C, N))) for b in range(B)]


_NC_CACHE = {}


def get_program(repeat: int = 1):
    if repeat not in _NC_CACHE:
        _NC_CACHE[repeat] = build_program(repeat)
    return _NC_CACHE[repeat]


def kernel(x, norm_w, norm_b, qkv_w, qkv_b, proj_w, proj_b):
    x = np.asarray(x, np.float32)
    B, C_, H_, W_ = x.shape
    in_maps = make_in_maps(x, norm_w, norm_b, qkv_w, qkv_b, proj_w, proj_b)
    nc = get_program()
    res = run_bass_kernel_spmd(nc, in_maps, core_ids=list(range(len(in_maps))))
    out = np.stack([np.asarray(res.results[b]["out"], np.float32) for b in range(B)])
    return out.reshape(B, C_, H_, W_)


# revision 5
# speedup vs baseline: 4.3506x; 4.3506x over previous
"""Trainium2 Bass kernel for nn_AttentionBlock (GroupNorm + spatial self-attention + residual).

Full inputs in, full outputs out. Internally: data-parallel over the batch dim
(B=8) across 8 NeuronCores; each core runs an identical Bass/Tile program on
one [C=256, N=4096] image.

Per-core layout choices:
  - q,k stored [c, n] (c on partitions, 2 chunks of 128)
  - v stored transposed [n, c] (computed directly by swapping matmul operand
    roles, no on-device transpose pass)
  - attention scores computed transposed: S'[j,i] = (K^T Q)[j,i], j on
    partitions, so the AV contraction over j runs as PSUM-accumulated matmuls
  - softmax denominator: DVE accumulation of exp tiles over j-tiles, then a
    ones-vector matmul to reduce the 128 partitions; normalization applied to
    A via a broadcast tile (ones outer-product matmul)
  - all big matmuls in bf16 (1 cycle/row on PE vs 4 for fp32); accumulation is
    always fp32 in PSUM. exp(s/16) runs on ACT straight out of PSUM with the
    1/sqrt(C) folded into the activation scale; no max-subtraction (scores are
    in [-8, 7] for this distribution, exp is safe in fp32).
"""

import sys

try:
    import concourse  # noqa: F401
except ImportError:
    sys.path.insert(0, "/opt/trn_rl_repo")

import numpy as np
import ml_dtypes

import concourse.bacc as bacc
import concourse.tile as tile
from concourse import mybir
from concourse.bass_utils import run_bass_kernel_spmd

F32 = mybir.dt.float32
BF16 = mybir.dt.bfloat16
AF = mybir.ActivationFunctionType
ALU = mybir.AluOpType
AX = mybir.AxisListType

C = 256          # channels
N = 4096         # spatial positions (64*64)
GROUPS = 32      # groupnorm groups -> 8 channels per group
EPS = 1e-5
SCALE = C ** -0.5
NSTRIPE = 8      # stripes over the spatial dim
SW = N // NSTRIPE  # 512
NJT = N // 128   # 32 j-tiles
GSIZE = (C // GROUPS) * N  # elements per group = 32768


def _emit(nc, tc, d):
    """Emit the per-core program. d: dict of DRAM tensor handles."""
    const = tc.alloc_tile_pool(name="const", bufs=1)

    # --- weights / params -> SBUF ---
    wq = []
    for t in range(2):
        w = const.tile([128, 768], BF16, tag=f"wq{t}", name=f"wq{t}")
        nc.sync.dma_start(w[:], d["wqkvT"][t * 128:(t + 1) * 128, :])
        wq.append(w)
    wp = []
    for t in range(2):
        w = const.tile([128, 256], BF16, tag=f"wp{t}", name=f"wp{t}")
        nc.sync.dma_start(w[:], d["wprojT"][t * 128:(t + 1) * 128, :])
        wp.append(w)
    vbb = const.tile([128, 256], BF16, tag="vbb")
    nc.sync.dma_start(vbb[:], d["vbias"][:])
    qkvb = const.tile([128, 4], F32, tag="qkvb")
    nc.sync.dma_start(qkvb[:], d["qkvb"][:])
    projb = const.tile([128, 2], F32, tag="projb")
    nc.sync.dma_start(projb[:], d["projb"][:])
    nw = const.tile([128, 2], F32, tag="nw")
    nc.sync.dma_start(nw[:], d["normw"][:])
    nb = const.tile([128, 2], F32, tag="nb")
    nc.sync.dma_start(nb[:], d["normb"][:])
    onesc = const.tile([128, 1], F32, tag="onesc")
    nc.sync.dma_start(onesc[:], d["onescol"][:])
    onesr = const.tile([1, 128], F32, tag="onesr")
    nc.sync.dma_start(onesr[:], d["onesrow"][:])
    gm = const.tile([128, 128], F32, tag="gm")
    nc.sync.dma_start(gm[:], d["gm"][:])

    # --- x -> SBUF (resident, also used for the residual at the end) ---
    xt = []
    for t in range(2):
        x_ = const.tile([128, N], F32, tag=f"x{t}", name=f"x{t}")
        nc.sync.dma_start(x_[:], d["x"][t * 128:(t + 1) * 128, :])
        xt.append(x_)

    # --- phase A: groupnorm stats -> per-channel scale/bias ---
    # stats cols: [sum(chunk0), sumsq(chunk0), sum(chunk1), sumsq(chunk1)]
    stats = const.tile([128, 4], F32, tag="stats")
    scl = const.tile([128, 2], F32, tag="scl")
    bia = const.tile([128, 2], F32, tag="bia")
    with tc.tile_pool(name="scratch", bufs=2) as scr, \
         tc.tile_pool(name="pa_ps", bufs=1, space="PSUM") as pa_ps:
        for t in range(2):
            nc.vector.reduce_sum(stats[:, 2 * t:2 * t + 1], xt[t][:], axis=AX.X)
            sq = scr.tile([128, N], F32, tag="sq")
            nc.scalar.activation(sq[:], xt[t][:], AF.Square,
                                 accum_out=stats[:, 2 * t + 1:2 * t + 2])
        gstats = pa_ps.tile([128, 4], F32, tag="gstats")
        nc.tensor.matmul(gstats[:], gm[:], stats[:], start=True, stop=True)

        mean = const.tile([128, 2], F32, tag="mean")
        ex2 = const.tile([128, 2], F32, tag="ex2")
        for t in range(2):
            nc.vector.tensor_scalar_mul(mean[:, t:t + 1], gstats[:, 2 * t:2 * t + 1], 1.0 / GSIZE)
            nc.vector.tensor_scalar_mul(ex2[:, t:t + 1], gstats[:, 2 * t + 1:2 * t + 2], 1.0 / GSIZE)
        m2 = const.tile([128, 2], F32, tag="m2")
        var = const.tile([128, 2], F32, tag="var")
        std = const.tile([128, 2], F32, tag="std")
        rstd = const.tile([128, 2], F32, tag="rstd")
        nc.vector.tensor_mul(m2[:], mean[:], mean[:])
        nc.vector.tensor_sub(var[:], ex2[:], m2[:])
        nc.vector.tensor_scalar_add(var[:], var[:], EPS)
        nc.scalar.activation(std[:], var[:], AF.Sqrt)
        nc.vector.reciprocal(rstd[:], std[:])
        nc.vector.tensor_mul(scl[:], nw[:], rstd[:])
        mscl = const.tile([128, 2], F32, tag="mscl")
        nc.vector.tensor_mul(mscl[:], mean[:], scl[:])
        nc.vector.tensor_sub(bia[:], nb[:], mscl[:])

    # --- phase B: h = x*scl+bia (bf16), q,k ([c,n]) and vT ([n,c]) ---
    qk = []
    for i in range(4):  # q0,q1,k0,k1
        t_ = const.tile([128, N], BF16, tag=f"qk{i}", name=f"qk{i}")
        qk.append(t_)
    vt = []
    for j in range(NJT):
        t_ = const.tile([128, 256], BF16, tag=f"vt{j}", name=f"vt{j}")
        vt.append(t_)

    with tc.tile_pool(name="hpool", bufs=3) as hp, \
         tc.tile_pool(name="pb_ps", bufs=3, space="PSUM") as pbp, \
         tc.tile_pool(name="pv_ps", bufs=3, space="PSUM") as pvp:
        for s in range(NSTRIPE):
            sl = slice(s * SW, (s + 1) * SW)
            hts = []
            for t in range(2):
                ht = hp.tile([128, SW], BF16, tag=f"h{t}", name=f"h{t}")
                nc.vector.tensor_scalar(ht[:], xt[t][:, sl], scl[:, t:t + 1],
                                        bia[:, t:t + 1], op0=ALU.mult, op1=ALU.add)
                hts.append(ht)
            for dt in range(4):
                ps = pbp.tile([128, SW], F32, tag="qkps", name="qkps")
                nc.tensor.matmul(ps[:], wq[0][:, dt * 128:(dt + 1) * 128], hts[0][:],
                                 start=True, stop=False)
                nc.tensor.matmul(ps[:], wq[1][:, dt * 128:(dt + 1) * 128], hts[1][:],
                                 start=False, stop=True)
                nc.vector.tensor_scalar_add(qk[dt][:, sl], ps[:], qkvb[:, dt:dt + 1])
            for n4 in range(4):
                jt = s * 4 + n4
                psv = pvp.tile([128, 256], F32, tag="vtps", name="vtps")
                nc.tensor.matmul(psv[:], hts[0][:, n4 * 128:(n4 + 1) * 128],
                                 wq[0][:, 512:768], start=True, stop=False)
                nc.tensor.matmul(psv[:], hts[1][:, n4 * 128:(n4 + 1) * 128],
                                 wq[1][:, 512:768], start=False, stop=True)
                nc.vector.tensor_add(vt[jt][:], psv[:], vbb[:])

    # --- phase C: attention + proj + residual, per i-stripe ---
    with tc.tile_pool(name="wpool", bufs=6) as wpo, \
         tc.tile_pool(name="raccp", bufs=2) as rp, \
         tc.tile_pool(name="misc", bufs=2) as mp, \
         tc.tile_pool(name="s_ps", bufs=2, space="PSUM") as sp, \
         tc.tile_pool(name="a_ps", bufs=2, space="PSUM") as apo, \
         tc.tile_pool(name="r_ps", bufs=1, space="PSUM") as rsp, \
         tc.tile_pool(name="b_ps", bufs=1, space="PSUM") as bpo, \
         tc.tile_pool(name="o_ps", bufs=2, space="PSUM") as opo:
        for ist in range(NSTRIPE):
            sl = slice(ist * SW, (ist + 1) * SW)
            racc = rp.tile([128, SW], F32, tag="racc")
            a_ps = [apo.tile([128, SW], F32, tag="aps", name="aps") for _ in range(2)]
            for jt in range(NJT):
                s_ps = sp.tile([128, SW], F32, tag="sps", name="sps")
                nc.tensor.matmul(s_ps[:], qk[2][:, jt * 128:(jt + 1) * 128], qk[0][:, sl],
                                 start=True, stop=False)
                nc.tensor.matmul(s_ps[:], qk[3][:, jt * 128:(jt + 1) * 128], qk[1][:, sl],
                                 start=False, stop=True)
                w_sb = wpo.tile([128, SW], BF16, tag="wsb", name="wsb")
                nc.scalar.activation(w_sb[:], s_ps[:], AF.Exp, scale=SCALE)
                if jt == 0:
                    nc.vector.tensor_copy(racc[:], w_sb[:])
                else:
                    nc.vector.tensor_add(racc[:], racc[:], w_sb[:])
                for ct in range(2):
                    nc.tensor.matmul(a_ps[ct][:], vt[jt][:, ct * 128:(ct + 1) * 128],
                                     w_sb[:], start=(jt == 0), stop=(jt == NJT - 1))
            rs_ps = rsp.tile([1, SW], F32, tag="rs")
            nc.tensor.matmul(rs_ps[:], onesc[:], racc[:], start=True, stop=True)
            rinv = mp.tile([1, SW], F32, tag="rinv")
            nc.vector.reciprocal(rinv[:], rs_ps[:])
            bc_ps = bpo.tile([128, SW], F32, tag="bc")
            nc.tensor.matmul(bc_ps[:], onesr[:], rinv[:], start=True, stop=True)
            bc_sb = mp.tile([128, SW], F32, tag="bcs")
            nc.vector.tensor_copy(bc_sb[:], bc_ps[:])
            a_sb = []
            for ct in range(2):
                t_ = mp.tile([128, SW], BF16, tag=f"asb{ct}", name=f"asb{ct}")
                nc.vector.tensor_mul(t_[:], a_ps[ct][:], bc_sb[:])
                a_sb.append(t_)
            for dt in range(2):
                o_ps = opo.tile([128, SW], F32, tag="ops", name="ops")
                nc.tensor.matmul(o_ps[:], wp[0][:, dt * 128:(dt + 1) * 128], a_sb[0][:],
                                 start=True, stop=False)
                nc.tensor.matmul(o_ps[:], wp[1][:, dt * 128:(dt + 1) * 128], a_sb[1][:],
                                 start=False, stop=True)
                o_sb = mp.tile([128, SW], F32, tag=f"osb{dt}", name=f"osb{dt}")
                nc.vector.scalar_tensor_tensor(o_sb[:], o_ps[:], projb[:, dt:dt + 1],
                                               xt[dt][:, sl], op0=ALU.add, op1=ALU.add)
                nc.sync.dma_start(d["out"][dt * 128:(dt + 1) * 128, sl], o_sb[:])

    const.release()


def build_program(repeat: int = 1):
    nc = bacc.Bacc("TRN2", target_bir_lowering=False, debug=False, num_devices=8)
    d = {
        "x": nc.declare_dram_parameter("x", [C, N], F32, isOutput=False),
        "wqkvT": nc.declare_dram_parameter("wqkvT", [C, 3 * C], BF16, isOutput=False),
        "wprojT": nc.declare_dram_parameter("wprojT", [C, C], BF16, isOutput=False),
        "qkvb": nc.declare_dram_parameter("qkvb", [128, 4], F32, isOutput=False),
        "vbias": nc.declare_dram_parameter("vbias", [128, 256], BF16, isOutput=False),
        "projb": nc.declare_dram_parameter("projb", [128, 2], F32, isOutput=False),
        "normw": nc.declare_dram_parameter("normw", [128, 2], F32, isOutput=False),
        "normb": nc.declare_dram_parameter("normb", [128, 2], F32, isOutput=False),
        "onescol": nc.declare_dram_parameter("onescol", [128, 1], F32, isOutput=False),
        "onesrow": nc.declare_dram_parameter("onesrow", [1, 128], F32, isOutput=False),
        "gm": nc.declare_dram_parameter("gm", [128, 128], F32, isOutput=False),
        "out": nc.declare_dram_parameter("out", [C, N], F32, isOutput=True),
    }
    with tile.TileContext(nc) as tc:
        for _ in range(repeat):
            _emit(nc, tc, d)
    nc.compile()
    return nc


def make_in_maps(x, norm_w, norm_b, qkv_w, qkv_b, proj_w, proj_b):
    x = np.asarray(x, np.float32)
    B = x.shape[0]
    qkv_w = np.asarray(qkv_w, np.float32)
    qkv_b = np.asarray(qkv_b, np.float32)
    proj_w = np.asarray(proj_w, np.float32)
    proj_b = np.asarray(proj_b, np.float32)
    shared = {
        "wqkvT": np.ascontiguousarray(qkv_w.T).astype(ml_dtypes.bfloat16),
        "wprojT": np.ascontiguousarray(proj_w.T).astype(ml_dtypes.bfloat16),
        "qkvb": np.ascontiguousarray(qkv_b[:512].reshape(4, 128).T),
        "vbias": np.tile(qkv_b[512:].reshape(1, 256), (128, 1)).astype(ml_dtypes.bfloat16),
        "projb": np.ascontiguousarray(proj_b.reshape(2, 128).T),
        "normw": np.ascontiguousarray(np.asarray(norm_w, np.float32).reshape(2, 128).T),
        "normb": np.ascontiguousarray(np.asarray(norm_b, np.float32).reshape(2, 128).T),
        "onescol": np.ones((128, 1), np.float32),
        "onesrow": np.ones((1, 128), np.float32),
        "gm": (np.arange(128)[:, None] // 8 == np.arange(128)[None, :] // 8).astype(np.float32),
    }
    return [dict(shared, x=np.ascontiguousarray(x[b].reshape(C, N))) for b in range(B)]


_NC_CACHE = {}


def get_program(repeat: int = 1):
    if repeat not in _NC_CACHE:
        _NC_CACHE[repeat] = build_program(repeat)
    return _NC_CACHE[repeat]


def kernel(x, norm_w, norm_b, qkv_w, qkv_b, proj_w, proj_b):
    x = np.asarray(x, np.float32)
    B, C_, H_, W_ = x.shape
    in_maps = make_in_maps(x, norm_w, norm_b, qkv_w, qkv_b, proj_w, proj_b)
    nc = get_program()
    res = run_bass_kernel_spmd(nc, in_maps, core_ids=list(range(len(in_maps))))
    out = np.stack([np.asarray(res.results[b]["out"], np.float32) for b in range(B)])
    return out.reshape(B, C_, H_, W_)


# revision 11
# speedup vs baseline: 17.4546x; 4.0120x over previous
"""Trainium2 Bass kernel for nn_AttentionBlock (GroupNorm + spatial self-attention + residual).

Full inputs in, full outputs out. Internally: data-parallel over the batch dim
(B=8) across 8 NeuronCores; each core runs an identical Bass/Tile program on
one [C=256, N=4096] image.

Per-core layout choices:
  - q,k stored [c, n] (c on partitions, 2 chunks of 128)
  - v stored transposed [n, c] (computed directly by swapping matmul operand
    roles, no on-device transpose pass)
  - attention scores computed transposed: S'[j,i] = (K^T Q)[j,i], j on
    partitions, so the AV contraction over j runs as PSUM-accumulated matmuls
  - softmax denominator: DVE accumulation of exp tiles over j-tiles, then a
    ones-vector matmul to reduce the 128 partitions; normalization applied to
    A via a broadcast tile (ones outer-product matmul)
  - all big matmuls in bf16 (1 cycle/row on PE vs 4 for fp32); accumulation is
    always fp32 in PSUM. exp(s/16) runs on ACT straight out of PSUM with the
    1/sqrt(C) folded into the activation scale; no max-subtraction (scores are
    in [-8, 7] for this distribution, exp is safe in fp32).
"""

import sys

try:
    import concourse  # noqa: F401
except ImportError:
    sys.path.insert(0, "/opt/trn_rl_repo")

import numpy as np
import ml_dtypes

import concourse.bacc as bacc
import concourse.tile as tile
from concourse import mybir
from concourse.bass_utils import run_bass_kernel_spmd

F32 = mybir.dt.float32
BF16 = mybir.dt.bfloat16
AF = mybir.ActivationFunctionType
ALU = mybir.AluOpType
AX = mybir.AxisListType

C = 256          # channels
N = 4096         # spatial positions (64*64)
GROUPS = 32      # groupnorm groups -> 8 channels per group
EPS = 1e-5
SCALE = C ** -0.5
NSTRIPE = 8      # stripes over the spatial dim
SW = N // NSTRIPE  # 512
NJT = N // 128   # 32 j-tiles
GSIZE = (C // GROUPS) * N  # elements per group = 32768


def _emit(nc, tc, d, parts="ABC"):
    """Emit the per-core program. d: dict of DRAM tensor handles."""
    const = tc.alloc_tile_pool(name="const", bufs=1)

    # --- weights / params -> SBUF ---
    wq = []
    for t in range(2):
        w = const.tile([128, 768], BF16, tag=f"wq{t}", name=f"wq{t}")
        nc.sync.dma_start(w[:], d["wqkvT"][t * 128:(t + 1) * 128, :])
        wq.append(w)
    wp = []
    for t in range(2):
        w = const.tile([128, 256], BF16, tag=f"wp{t}", name=f"wp{t}")
        nc.sync.dma_start(w[:], d["wprojT"][t * 128:(t + 1) * 128, :])
        wp.append(w)
    vbb = const.tile([128, 256], BF16, tag="vbb")
    nc.sync.dma_start(vbb[:], d["vbias"][:])
    qkvb = const.tile([128, 4], F32, tag="qkvb")
    nc.sync.dma_start(qkvb[:], d["qkvb"][:])
    projb = const.tile([128, 2], F32, tag="projb")
    nc.sync.dma_start(projb[:], d["projb"][:])
    nw = const.tile([128, 2], F32, tag="nw")
    nc.sync.dma_start(nw[:], d["normw"][:])
    nb = const.tile([128, 2], F32, tag="nb")
    nc.sync.dma_start(nb[:], d["normb"][:])
    onesc = const.tile([128, 1], F32, tag="onesc")
    nc.sync.dma_start(onesc[:], d["onescol"][:])
    onesr = const.tile([1, 128], F32, tag="onesr")
    nc.sync.dma_start(onesr[:], d["onesrow"][:])
    gm = const.tile([128, 128], F32, tag="gm")
    nc.sync.dma_start(gm[:], d["gm"][:])

    # --- x -> SBUF (resident, also used for the residual at the end) ---
    xt = []
    for t in range(2):
        x_ = const.tile([128, N], F32, tag=f"x{t}", name=f"x{t}")
        nc.sync.dma_start(x_[:], d["x"][t * 128:(t + 1) * 128, :])
        xt.append(x_)

    # --- phase A: groupnorm stats -> per-channel scale/bias ---
    # stats cols: [sum(chunk0), sumsq(chunk0), sum(chunk1), sumsq(chunk1)]
    stats = const.tile([128, 4], F32, tag="stats")
    scl = const.tile([128, 2], F32, tag="scl")
    bia = const.tile([128, 2], F32, tag="bia")
    with tc.tile_pool(name="scratch", bufs=2) as scr, \
         tc.tile_pool(name="pa_ps", bufs=1, space="PSUM") as pa_ps:
        for t in range(2):
            nc.vector.reduce_sum(stats[:, 2 * t:2 * t + 1], xt[t][:], axis=AX.X)
            sq = scr.tile([128, N], F32, tag="sq")
            nc.scalar.activation(sq[:], xt[t][:], AF.Square,
                                 accum_out=stats[:, 2 * t + 1:2 * t + 2])
        gstats = pa_ps.tile([128, 4], F32, tag="gstats")
        nc.tensor.matmul(gstats[:], gm[:], stats[:], start=True, stop=True)

        mean = const.tile([128, 2], F32, tag="mean")
        ex2 = const.tile([128, 2], F32, tag="ex2")
        for t in range(2):
            nc.vector.tensor_scalar_mul(mean[:, t:t + 1], gstats[:, 2 * t:2 * t + 1], 1.0 / GSIZE)
            nc.vector.tensor_scalar_mul(ex2[:, t:t + 1], gstats[:, 2 * t + 1:2 * t + 2], 1.0 / GSIZE)
        m2 = const.tile([128, 2], F32, tag="m2")
        var = const.tile([128, 2], F32, tag="var")
        std = const.tile([128, 2], F32, tag="std")
        rstd = const.tile([128, 2], F32, tag="rstd")
        nc.vector.tensor_mul(m2[:], mean[:], mean[:])
        nc.vector.tensor_sub(var[:], ex2[:], m2[:])
        nc.vector.tensor_scalar_add(var[:], var[:], EPS)
        nc.scalar.activation(std[:], var[:], AF.Sqrt)
        nc.vector.reciprocal(rstd[:], std[:])
        nc.vector.tensor_mul(scl[:], nw[:], rstd[:])
        mscl = const.tile([128, 2], F32, tag="mscl")
        nc.vector.tensor_mul(mscl[:], mean[:], scl[:])
        nc.vector.tensor_sub(bia[:], nb[:], mscl[:])

    # --- phase B: h = x*scl+bia (bf16), q,k ([c,n]) and vT ([n,c]) ---
    qk = []
    for i in range(4):  # q0,q1,k0,k1
        t_ = const.tile([128, N], BF16, tag=f"qk{i}", name=f"qk{i}")
        qk.append(t_)
    vt = []
    for j in range(NJT):
        t_ = const.tile([128, 256], BF16, tag=f"vt{j}", name=f"vt{j}")
        vt.append(t_)

    with tc.tile_pool(name="hpool", bufs=3) as hp, \
         tc.tile_pool(name="pb_ps", bufs=3, space="PSUM") as pbp, \
         tc.tile_pool(name="pv_ps", bufs=3, space="PSUM") as pvp:
        for s in range(NSTRIPE):
            sl = slice(s * SW, (s + 1) * SW)
            hts = []
            for t in range(2):
                ht = hp.tile([128, SW], BF16, tag=f"h{t}", name=f"h{t}")
                nc.vector.tensor_scalar(ht[:], xt[t][:, sl], scl[:, t:t + 1],
                                        bia[:, t:t + 1], op0=ALU.mult, op1=ALU.add)
                hts.append(ht)
            for dt in range(4):
                ps = pbp.tile([128, SW], F32, tag="qkps", name="qkps")
                nc.tensor.matmul(ps[:], wq[0][:, dt * 128:(dt + 1) * 128], hts[0][:],
                                 start=True, stop=False)
                nc.tensor.matmul(ps[:], wq[1][:, dt * 128:(dt + 1) * 128], hts[1][:],
                                 start=False, stop=True)
                nc.vector.tensor_scalar_add(qk[dt][:, sl], ps[:], qkvb[:, dt:dt + 1])
            for n4 in range(4):
                jt = s * 4 + n4
                psv = pvp.tile([128, 256], F32, tag="vtps", name="vtps")
                nc.tensor.matmul(psv[:], hts[0][:, n4 * 128:(n4 + 1) * 128],
                                 wq[0][:, 512:768], start=True, stop=False)
                nc.tensor.matmul(psv[:], hts[1][:, n4 * 128:(n4 + 1) * 128],
                                 wq[1][:, 512:768], start=False, stop=True)
                nc.vector.tensor_add(vt[jt][:], psv[:], vbb[:])

    # --- phase C: attention + proj + residual, per i-stripe ---
    if "C" not in parts:
        # timing variant: still write something to out so nothing is elided
        dummy = const.tile([128, 16], F32, tag="dummy")
        nc.vector.tensor_copy(dummy[:], xt[0][:, 0:16])
        nc.sync.dma_start(d["out"][0:128, 0:16], dummy[:])
        const.release()
        return
    with tc.tile_pool(name="wpool", bufs=6) as wpo, \
         tc.tile_pool(name="raccp", bufs=2) as rp, \
         tc.tile_pool(name="misc", bufs=2) as mp, \
         tc.tile_pool(name="s_ps", bufs=2, space="PSUM") as sp, \
         tc.tile_pool(name="a_ps", bufs=2, space="PSUM") as apo, \
         tc.tile_pool(name="r_ps", bufs=1, space="PSUM") as rsp, \
         tc.tile_pool(name="b_ps", bufs=1, space="PSUM") as bpo, \
         tc.tile_pool(name="o_ps", bufs=2, space="PSUM") as opo:
        for ist in range(NSTRIPE):
            sl = slice(ist * SW, (ist + 1) * SW)
            racc = rp.tile([128, SW], F32, tag="racc")
            a_ps = [apo.tile([128, SW], F32, tag="aps", name="aps") for _ in range(2)]
            for jt in range(NJT):
                s_ps = sp.tile([128, SW], F32, tag="sps", name="sps")
                nc.tensor.matmul(s_ps[:], qk[2][:, jt * 128:(jt + 1) * 128], qk[0][:, sl],
                                 start=True, stop=False)
                nc.tensor.matmul(s_ps[:], qk[3][:, jt * 128:(jt + 1) * 128], qk[1][:, sl],
                                 start=False, stop=True)
                w_sb = wpo.tile([128, SW], BF16, tag="wsb", name="wsb")
                nc.scalar.activation(w_sb[:], s_ps[:], AF.Exp, scale=SCALE)
                if jt == 0:
                    nc.vector.tensor_copy(racc[:], w_sb[:])
                else:
                    nc.vector.tensor_add(racc[:], racc[:], w_sb[:])
                if "noav" in parts:
                    continue
                for ct in range(2):
                    nc.tensor.matmul(a_ps[ct][:], vt[jt][:, ct * 128:(ct + 1) * 128],
                                     w_sb[:], start=(jt == 0), stop=(jt == NJT - 1))
            if "noav" in parts:
                o_sb = mp.tile([128, SW], F32, tag="osb0", name="osb0")
                nc.vector.tensor_add(o_sb[:], racc[:], xt[0][:, sl])
                nc.sync.dma_start(d["out"][0:128, sl], o_sb[:])
                continue
            rs_ps = rsp.tile([1, SW], F32, tag="rs")
            nc.tensor.matmul(rs_ps[:], onesc[:], racc[:], start=True, stop=True)
            rinv = mp.tile([1, SW], F32, tag="rinv")
            nc.vector.reciprocal(rinv[:], rs_ps[:])
            bc_ps = bpo.tile([128, SW], F32, tag="bc")
            nc.tensor.matmul(bc_ps[:], onesr[:], rinv[:], start=True, stop=True)
            bc_sb = mp.tile([128, SW], F32, tag="bcs")
            nc.vector.tensor_copy(bc_sb[:], bc_ps[:])
            a_sb = []
            for ct in range(2):
                t_ = mp.tile([128, SW], BF16, tag=f"asb{ct}", name=f"asb{ct}")
                nc.vector.tensor_mul(t_[:], a_ps[ct][:], bc_sb[:])
                a_sb.append(t_)
            for dt in range(2):
                o_ps = opo.tile([128, SW], F32, tag="ops", name="ops")
                nc.tensor.matmul(o_ps[:], wp[0][:, dt * 128:(dt + 1) * 128], a_sb[0][:],
                                 start=True, stop=False)
                nc.tensor.matmul(o_ps[:], wp[1][:, dt * 128:(dt + 1) * 128], a_sb[1][:],
                                 start=False, stop=True)
                o_sb = mp.tile([128, SW], F32, tag=f"osb{dt}", name=f"osb{dt}")
                nc.vector.scalar_tensor_tensor(o_sb[:], o_ps[:], projb[:, dt:dt + 1],
                                               xt[dt][:, sl], op0=ALU.add, op1=ALU.add)
                nc.sync.dma_start(d["out"][dt * 128:(dt + 1) * 128, sl], o_sb[:])

    const.release()


def build_program(repeat: int = 1, parts: str = "ABC"):
    nc = bacc.Bacc("TRN2", target_bir_lowering=False, debug=False, num_devices=8)
    d = {
        "x": nc.declare_dram_parameter("x", [C, N], F32, isOutput=False),
        "wqkvT": nc.declare_dram_parameter("wqkvT", [C, 3 * C], BF16, isOutput=False),
        "wprojT": nc.declare_dram_parameter("wprojT", [C, C], BF16, isOutput=False),
        "qkvb": nc.declare_dram_parameter("qkvb", [128, 4], F32, isOutput=False),
        "vbias": nc.declare_dram_parameter("vbias", [128, 256], BF16, isOutput=False),
        "projb": nc.declare_dram_parameter("projb", [128, 2], F32, isOutput=False),
        "normw": nc.declare_dram_parameter("normw", [128, 2], F32, isOutput=False),
        "normb": nc.declare_dram_parameter("normb", [128, 2], F32, isOutput=False),
        "onescol": nc.declare_dram_parameter("onescol", [128, 1], F32, isOutput=False),
        "onesrow": nc.declare_dram_parameter("onesrow", [1, 128], F32, isOutput=False),
        "gm": nc.declare_dram_parameter("gm", [128, 128], F32, isOutput=False),
        "out": nc.declare_dram_parameter("out", [C, N], F32, isOutput=True),
    }
    with tile.TileContext(nc) as tc:
        for _ in range(repeat):
            _emit(nc, tc, d, parts)
    nc.compile()
    return nc


def make_in_maps(x, norm_w, norm_b, qkv_w, qkv_b, proj_w, proj_b):
    x = np.asarray(x, np.float32)
    B = x.shape[0]
    qkv_w = np.asarray(qkv_w, np.float32)
    qkv_b = np.asarray(qkv_b, np.float32)
    proj_w = np.asarray(proj_w, np.float32)
    proj_b = np.asarray(proj_b, np.float32)
    shared = {
        "wqkvT": np.ascontiguousarray(qkv_w.T).astype(ml_dtypes.bfloat16),
        "wprojT": np.ascontiguousarray(proj_w.T).astype(ml_dtypes.bfloat16),
        "qkvb": np.ascontiguousarray(qkv_b[:512].reshape(4, 128).T),
        "vbias": np.tile(qkv_b[512:].reshape(1, 256), (128, 1)).astype(ml_dtypes.bfloat16),
        "projb": np.ascontiguousarray(proj_b.reshape(2, 128).T),
        "normw": np.ascontiguousarray(np.asarray(norm_w, np.float32).reshape(2, 128).T),
        "normb": np.ascontiguousarray(np.asarray(norm_b, np.float32).reshape(2, 128).T),
        "onescol": np.ones((128, 1), np.float32),
        "onesrow": np.ones((1, 128), np.float32),
        "gm": (np.arange(128)[:, None] // 8 == np.arange(128)[None, :] // 8).astype(np.float32),
    }
    return [dict(shared, x=np.ascontiguousarray(x[b].reshape(C, N))) for b in range(B)]


_NC_CACHE = {}


def get_program(repeat: int = 1):
    if repeat not in _NC_CACHE:
        _NC_CACHE[repeat] = build_program(repeat)
    return _NC_CACHE[repeat]


def kernel(x, norm_w, norm_b, qkv_w, qkv_b, proj_w, proj_b):
    x = np.asarray(x, np.float32)
    B, C_, H_, W_ = x.shape
    in_maps = make_in_maps(x, norm_w, norm_b, qkv_w, qkv_b, proj_w, proj_b)
    nc = get_program()
    res = run_bass_kernel_spmd(nc, in_maps, core_ids=list(range(len(in_maps))))
    out = np.stack([np.asarray(res.results[b]["out"], np.float32) for b in range(B)])
    return out.reshape(B, C_, H_, W_)


# revision 14
# speedup vs baseline: 108.3135x; 6.2054x over previous
"""Trainium2 Bass kernel for nn_AttentionBlock (GroupNorm + spatial self-attention + residual).

Full inputs in, full outputs out. Internally: data-parallel over the batch dim
(B=8) across 8 NeuronCores; each core runs an identical Bass/Tile program on
one [C=256, N=4096] image.

Per-core layout choices:
  - q,k stored [c, n] (c on partitions, 2 chunks of 128)
  - v stored transposed [n, c] (computed directly by swapping matmul operand
    roles, no on-device transpose pass)
  - attention scores computed transposed: S'[j,i] = (K^T Q)[j,i], j on
    partitions, so the AV contraction over j runs as PSUM-accumulated matmuls
  - softmax denominator: DVE accumulation of exp tiles over j-tiles, then a
    ones-vector matmul to reduce the 128 partitions; normalization applied to
    A via a broadcast tile (ones outer-product matmul)
  - all big matmuls in bf16 (1 cycle/row on PE vs 4 for fp32); accumulation is
    always fp32 in PSUM. exp(s/16) runs on ACT straight out of PSUM with the
    1/sqrt(C) folded into the activation scale; no max-subtraction (scores are
    in [-8, 7] for this distribution, exp is safe in fp32).
"""

import sys

try:
    import concourse  # noqa: F401
except ImportError:
    sys.path.insert(0, "/opt/trn_rl_repo")

import numpy as np
import ml_dtypes

import concourse.bacc as bacc
import concourse.tile as tile
from concourse import mybir
from concourse.bass_utils import run_bass_kernel_spmd

F32 = mybir.dt.float32
BF16 = mybir.dt.bfloat16
AF = mybir.ActivationFunctionType
ALU = mybir.AluOpType
AX = mybir.AxisListType

C = 256          # channels
N = 4096         # spatial positions (64*64)
GROUPS = 32      # groupnorm groups -> 8 channels per group
EPS = 1e-5
SCALE = C ** -0.5
NSTRIPE = 8      # stripes over the spatial dim
SW = N // NSTRIPE  # 512
NJT = N // 128   # 32 j-tiles
GSIZE = (C // GROUPS) * N  # elements per group = 32768


def _emit(nc, tc, d, parts="ABC"):
    """Emit the per-core program. d: dict of DRAM tensor handles."""
    const = tc.alloc_tile_pool(name="const", bufs=1)

    # --- weights / params -> SBUF ---
    wq = []
    for t in range(2):
        w = const.tile([128, 768], BF16, tag=f"wq{t}", name=f"wq{t}")
        nc.sync.dma_start(w[:], d["wqkvT"][t * 128:(t + 1) * 128, :])
        wq.append(w)
    wp = []
    for t in range(2):
        w = const.tile([128, 256], BF16, tag=f"wp{t}", name=f"wp{t}")
        nc.sync.dma_start(w[:], d["wprojT"][t * 128:(t + 1) * 128, :])
        wp.append(w)
    vbb = const.tile([128, 256], BF16, tag="vbb")
    nc.sync.dma_start(vbb[:], d["vbias"][:])
    qkvb = const.tile([128, 4], F32, tag="qkvb")
    nc.sync.dma_start(qkvb[:], d["qkvb"][:])
    projb = const.tile([128, 2], F32, tag="projb")
    nc.sync.dma_start(projb[:], d["projb"][:])
    nw = const.tile([128, 2], F32, tag="nw")
    nc.sync.dma_start(nw[:], d["normw"][:])
    nb = const.tile([128, 2], F32, tag="nb")
    nc.sync.dma_start(nb[:], d["normb"][:])
    onesc = const.tile([128, 1], F32, tag="onesc")
    nc.sync.dma_start(onesc[:], d["onescol"][:])
    onesr = const.tile([1, 128], F32, tag="onesr")
    nc.sync.dma_start(onesr[:], d["onesrow"][:])
    gm = const.tile([128, 128], F32, tag="gm")
    nc.sync.dma_start(gm[:], d["gm"][:])

    # --- x -> SBUF (resident, also used for the residual at the end) ---
    xt = []
    for t in range(2):
        x_ = const.tile([128, N], F32, tag=f"x{t}", name=f"x{t}")
        nc.sync.dma_start(x_[:], d["x"][t * 128:(t + 1) * 128, :])
        xt.append(x_)

    # --- phase A: groupnorm stats -> per-channel scale/bias ---
    # stats cols: [sum(chunk0), sumsq(chunk0), sum(chunk1), sumsq(chunk1)]
    stats = const.tile([128, 4], F32, tag="stats")
    scl = const.tile([128, 2], F32, tag="scl")
    bia = const.tile([128, 2], F32, tag="bia")
    with tc.tile_pool(name="scratch", bufs=2) as scr, \
         tc.tile_pool(name="pa_ps", bufs=1, space="PSUM") as pa_ps:
        for t in range(2):
            nc.vector.reduce_sum(stats[:, 2 * t:2 * t + 1], xt[t][:], axis=AX.X)
            sq = scr.tile([128, N], F32, tag="sq")
            nc.scalar.activation(sq[:], xt[t][:], AF.Square,
                                 accum_out=stats[:, 2 * t + 1:2 * t + 2])
        gstats = pa_ps.tile([128, 4], F32, tag="gstats")
        nc.tensor.matmul(gstats[:], gm[:], stats[:], start=True, stop=True)

        mean = const.tile([128, 2], F32, tag="mean")
        ex2 = const.tile([128, 2], F32, tag="ex2")
        for t in range(2):
            nc.vector.tensor_scalar_mul(mean[:, t:t + 1], gstats[:, 2 * t:2 * t + 1], 1.0 / GSIZE)
            nc.vector.tensor_scalar_mul(ex2[:, t:t + 1], gstats[:, 2 * t + 1:2 * t + 2], 1.0 / GSIZE)
        m2 = const.tile([128, 2], F32, tag="m2")
        var = const.tile([128, 2], F32, tag="var")
        std = const.tile([128, 2], F32, tag="std")
        rstd = const.tile([128, 2], F32, tag="rstd")
        nc.vector.tensor_mul(m2[:], mean[:], mean[:])
        nc.vector.tensor_sub(var[:], ex2[:], m2[:])
        nc.vector.tensor_scalar_add(var[:], var[:], EPS)
        nc.scalar.activation(std[:], var[:], AF.Sqrt)
        nc.vector.reciprocal(rstd[:], std[:])
        nc.vector.tensor_mul(scl[:], nw[:], rstd[:])
        mscl = const.tile([128, 2], F32, tag="mscl")
        nc.vector.tensor_mul(mscl[:], mean[:], scl[:])
        nc.vector.tensor_sub(bia[:], nb[:], mscl[:])

    # --- phase B: h = x*scl+bia (bf16), q,k ([c,n]) and vT ([n,c]) ---
    qk = []
    for i in range(4):  # q0,q1,k0,k1
        t_ = const.tile([128, N], BF16, tag=f"qk{i}", name=f"qk{i}")
        qk.append(t_)
    vt = []
    for j in range(NJT):
        t_ = const.tile([128, 256], BF16, tag=f"vt{j}", name=f"vt{j}")
        vt.append(t_)

    with tc.tile_pool(name="hpool", bufs=3) as hp, \
         tc.tile_pool(name="pb_ps", bufs=3, space="PSUM") as pbp, \
         tc.tile_pool(name="pv_ps", bufs=3, space="PSUM") as pvp:
        for s in range(NSTRIPE):
            sl = slice(s * SW, (s + 1) * SW)
            hts = []
            for t in range(2):
                ht = hp.tile([128, SW], BF16, tag=f"h{t}", name=f"h{t}")
                nc.vector.tensor_scalar(ht[:], xt[t][:, sl], scl[:, t:t + 1],
                                        bia[:, t:t + 1], op0=ALU.mult, op1=ALU.add)
                hts.append(ht)
            for dt in range(4):
                ps = pbp.tile([128, SW], F32, tag="qkps", name="qkps")
                nc.tensor.matmul(ps[:], wq[0][:, dt * 128:(dt + 1) * 128], hts[0][:],
                                 start=True, stop=False)
                nc.tensor.matmul(ps[:], wq[1][:, dt * 128:(dt + 1) * 128], hts[1][:],
                                 start=False, stop=True)
                nc.vector.tensor_scalar_add(qk[dt][:, sl], ps[:], qkvb[:, dt:dt + 1])
            for n4 in range(4):
                jt = s * 4 + n4
                psv = pvp.tile([128, 256], F32, tag="vtps", name="vtps")
                nc.tensor.matmul(psv[:], hts[0][:, n4 * 128:(n4 + 1) * 128],
                                 wq[0][:, 512:768], start=True, stop=False)
                nc.tensor.matmul(psv[:], hts[1][:, n4 * 128:(n4 + 1) * 128],
                                 wq[1][:, 512:768], start=False, stop=True)
                nc.vector.tensor_add(vt[jt][:], psv[:], vbb[:])

    # --- phase C: attention + proj + residual, per i-stripe ---
    if "C" not in parts:
        # timing variant: still write something to out so nothing is elided
        dummy = const.tile([128, 16], F32, tag="dummy")
        nc.vector.tensor_copy(dummy[:], xt[0][:, 0:16])
        nc.sync.dma_start(d["out"][0:128, 0:16], dummy[:])
        const.release()
        return
    with tc.tile_pool(name="wpool", bufs=9) as wpo, \
         tc.tile_pool(name="raccp", bufs=2) as rp, \
         tc.tile_pool(name="misc", bufs=2) as mp, \
         tc.tile_pool(name="s_ps", bufs=3, space="PSUM") as sp, \
         tc.tile_pool(name="a_ps", bufs=2, space="PSUM") as apo, \
         tc.tile_pool(name="r_ps", bufs=1, space="PSUM") as rsp, \
         tc.tile_pool(name="b_ps", bufs=1, space="PSUM") as bpo, \
         tc.tile_pool(name="o_ps", bufs=1, space="PSUM") as opo:
        for ist in range(NSTRIPE):
            sl = slice(ist * SW, (ist + 1) * SW)
            racc = rp.tile([128, SW], F32, tag="racc")
            a_ps = [apo.tile([128, SW], F32, tag="aps", name="aps") for _ in range(2)]
            # Software-pipeline: AV matmuls run LAG steps behind the S'/exp
            # production so the (in-order) PE queue never head-of-line blocks
            # on a cross-engine exp dependency.
            LAG = 6
            w_tiles = {}
            for jt in range(NJT + LAG):
                if jt < NJT:
                    s_ps = sp.tile([128, SW], F32, tag="sps", name="sps")
                    nc.tensor.matmul(s_ps[:], qk[2][:, jt * 128:(jt + 1) * 128], qk[0][:, sl],
                                     start=True, stop=False)
                    nc.tensor.matmul(s_ps[:], qk[3][:, jt * 128:(jt + 1) * 128], qk[1][:, sl],
                                     start=False, stop=True)
                    w_sb = wpo.tile([128, SW], BF16, tag="wsb", name="wsb")
                    nc.scalar.activation(w_sb[:], s_ps[:], AF.Exp, scale=SCALE)
                    if jt == 0:
                        nc.vector.tensor_copy(racc[:], w_sb[:])
                    else:
                        nc.vector.tensor_add(racc[:], racc[:], w_sb[:])
                    w_tiles[jt] = w_sb
                if "noav" in parts:
                    continue
                if jt >= LAG:
                    j2 = jt - LAG
                    w2 = w_tiles.pop(j2)
                    for ct in range(2):
                        nc.tensor.matmul(a_ps[ct][:], vt[j2][:, ct * 128:(ct + 1) * 128],
                                         w2[:], start=(j2 == 0), stop=(j2 == NJT - 1))
            if "noav" in parts:
                o_sb = mp.tile([128, SW], F32, tag="osb0", name="osb0")
                nc.vector.tensor_add(o_sb[:], racc[:], xt[0][:, sl])
                nc.sync.dma_start(d["out"][0:128, sl], o_sb[:])
                continue
            rs_ps = rsp.tile([1, SW], F32, tag="rs")
            nc.tensor.matmul(rs_ps[:], onesc[:], racc[:], start=True, stop=True)
            rinv = mp.tile([1, SW], F32, tag="rinv")
            nc.vector.reciprocal(rinv[:], rs_ps[:])
            bc_ps = bpo.tile([128, SW], F32, tag="bc")
            nc.tensor.matmul(bc_ps[:], onesr[:], rinv[:], start=True, stop=True)
            bc_sb = mp.tile([128, SW], F32, tag="bcs")
            nc.vector.tensor_copy(bc_sb[:], bc_ps[:])
            a_sb = []
            for ct in range(2):
                t_ = mp.tile([128, SW], BF16, tag=f"asb{ct}", name=f"asb{ct}")
                nc.vector.tensor_mul(t_[:], a_ps[ct][:], bc_sb[:])
                a_sb.append(t_)
            for dt in range(2):
                o_ps = opo.tile([128, SW], F32, tag="ops", name="ops")
                nc.tensor.matmul(o_ps[:], wp[0][:, dt * 128:(dt + 1) * 128], a_sb[0][:],
                                 start=True, stop=False)
                nc.tensor.matmul(o_ps[:], wp[1][:, dt * 128:(dt + 1) * 128], a_sb[1][:],
                                 start=False, stop=True)
                o_sb = mp.tile([128, SW], F32, tag=f"osb{dt}", name=f"osb{dt}")
                nc.vector.scalar_tensor_tensor(o_sb[:], o_ps[:], projb[:, dt:dt + 1],
                                               xt[dt][:, sl], op0=ALU.add, op1=ALU.add)
                nc.sync.dma_start(d["out"][dt * 128:(dt + 1) * 128, sl], o_sb[:])

    const.release()


def build_program(repeat: int = 1, parts: str = "ABC"):
    nc = bacc.Bacc("TRN2", target_bir_lowering=False, debug=False, num_devices=8)
    d = {
        "x": nc.declare_dram_parameter("x", [C, N], F32, isOutput=False),
        "wqkvT": nc.declare_dram_parameter("wqkvT", [C, 3 * C], BF16, isOutput=False),
        "wprojT": nc.declare_dram_parameter("wprojT", [C, C], BF16, isOutput=False),
        "qkvb": nc.declare_dram_parameter("qkvb", [128, 4], F32, isOutput=False),
        "vbias": nc.declare_dram_parameter("vbias", [128, 256], BF16, isOutput=False),
        "projb": nc.declare_dram_parameter("projb", [128, 2], F32, isOutput=False),
        "normw": nc.declare_dram_parameter("normw", [128, 2], F32, isOutput=False),
        "normb": nc.declare_dram_parameter("normb", [128, 2], F32, isOutput=False),
        "onescol": nc.declare_dram_parameter("onescol", [128, 1], F32, isOutput=False),
        "onesrow": nc.declare_dram_parameter("onesrow", [1, 128], F32, isOutput=False),
        "gm": nc.declare_dram_parameter("gm", [128, 128], F32, isOutput=False),
        "out": nc.declare_dram_parameter("out", [C, N], F32, isOutput=True),
    }
    with tile.TileContext(nc) as tc:
        for _ in range(repeat):
            _emit(nc, tc, d, parts)
    nc.compile()
    return nc


def make_in_maps(x, norm_w, norm_b, qkv_w, qkv_b, proj_w, proj_b):
    x = np.asarray(x, np.float32)
    B = x.shape[0]
    qkv_w = np.asarray(qkv_w, np.float32)
    qkv_b = np.asarray(qkv_b, np.float32)
    proj_w = np.asarray(proj_w, np.float32)
    proj_b = np.asarray(proj_b, np.float32)
    shared = {
        "wqkvT": np.ascontiguousarray(qkv_w.T).astype(ml_dtypes.bfloat16),
        "wprojT": np.ascontiguousarray(proj_w.T).astype(ml_dtypes.bfloat16),
        "qkvb": np.ascontiguousarray(qkv_b[:512].reshape(4, 128).T),
        "vbias": np.tile(qkv_b[512:].reshape(1, 256), (128, 1)).astype(ml_dtypes.bfloat16),
        "projb": np.ascontiguousarray(proj_b.reshape(2, 128).T),
        "normw": np.ascontiguousarray(np.asarray(norm_w, np.float32).reshape(2, 128).T),
        "normb": np.ascontiguousarray(np.asarray(norm_b, np.float32).reshape(2, 128).T),
        "onescol": np.ones((128, 1), np.float32),
        "onesrow": np.ones((1, 128), np.float32),
        "gm": (np.arange(128)[:, None] // 8 == np.arange(128)[None, :] // 8).astype(np.float32),
    }
    return [dict(shared, x=np.ascontiguousarray(x[b].reshape(C, N))) for b in range(B)]


_NC_CACHE = {}


def get_program(repeat: int = 1):
    if repeat not in _NC_CACHE:
        _NC_CACHE[repeat] = build_program(repeat)
    return _NC_CACHE[repeat]


def kernel(x, norm_w, norm_b, qkv_w, qkv_b, proj_w, proj_b):
    x = np.asarray(x, np.float32)
    B, C_, H_, W_ = x.shape
    in_maps = make_in_maps(x, norm_w, norm_b, qkv_w, qkv_b, proj_w, proj_b)
    nc = get_program()
    res = run_bass_kernel_spmd(nc, in_maps, core_ids=list(range(len(in_maps))))
    out = np.stack([np.asarray(res.results[b]["out"], np.float32) for b in range(B)])
    return out.reshape(B, C_, H_, W_)
